# revision 16
# baseline (speedup 1.0000x reference)
"""ArcFace (AngularPenaltySMLoss) Trainium2 kernel.

Computes, for x [N, D], W [C, D], labels [N]:
  xn = x / max(||x||_2, 1e-12)   (row-normalize)
  wf = xn @ W.T                  [N, C]
  target = wf[i, labels[i]]
  numerator = S * cos(arccos(clip(target)) + M)
  L = numerator - log(exp(numerator) + sum_{j != label} exp(S * wf[i, j]))
  returns (wf, -mean(L))

Distribution: W is sharded over the class dim C across 8 NeuronCores
(tensor-parallel margin-softmax). Each core computes its wf shard plus
per-row sums of exp(S*wf) over its shard; the [N]-sized combination
(label gather, margin, log) runs on host.

Layout/precision strategy:
  - Host normalizes x (f32, same formula as the reference), casts to
    bf16 and pre-transposes to xnT [D, N]; host also pre-transposes +
    bf16-casts each W shard to wT [D, CS]. The device then needs zero
    transposes: the contraction dim D lands on partitions from DMA.
  - Matmuls run bf16 x bf16 -> f32 PSUM at full PE rate (fp32r is a
    half-rate LOW_HIGH two-pass mode on trn2; fp32 is quarter-rate).
  - Class blocks are processed 4-at-a-time (2048 classes / 4 PSUM
    banks) so the PSUM->bf16 cast (DVE) and exp+row-sum (ACT) pay
    their fixed per-instruction cost once per 2048 elements.
  - wf is stored/DMA'd as bf16 (the output gate is scale-relative
    2e-2; bf16 rounding lands ~4e-3 max here). exp(64*wf) runs on the
    scalar engine from the bf16 tile at full 16-bit rate with per-row
    f32 accumulation; the [1024]-sized finishing math runs on host.
"""

import os
import sys

import numpy as np

for _p in ("/opt/trn_rl_repo", "/root/.axon_site/_ro/trn_rl_repo"):
    if os.path.isdir(_p) and _p not in sys.path:
        sys.path.insert(0, _p)

import ml_dtypes

import concourse.bass as bass
import concourse.tile as tile
from concourse import bacc, mybir
from concourse.bass_utils import run_bass_kernel_spmd

AF = mybir.ActivationFunctionType
F32 = mybir.dt.float32
BF16 = mybir.dt.bfloat16

S_SCALE = 64.0
MARGIN = 0.5
CLIP_EPS = 1e-7

N_CORES = 8
P = 128
CB = 512  # class-block width (one PSUM bank of fp32)
QB = 4  # class blocks per super-chunk (4 PSUM banks)

# Problem dims (hardcoded; kernel() asserts against them)
FULL_N, FULL_D, FULL_C = 1024, 512, 85742
CS = 10752  # per-core class shard, padded: 8 * 10752 = 86016 = 21 blocks of 512


def _chunks(ncb):
    # First chunk small-ish (its W DMA gates the very first matmul),
    # tail chunk smaller than QB (shorter exp on the drain chain).
    sizes = []
    first = min(2, ncb)
    sizes.append(first)
    rem = ncb - first
    while rem > 0:
        if rem > QB:
            sizes.append(QB)
            rem -= QB
        else:
            sizes.append(rem)
            rem = 0
    out = []
    c0 = 0
    for s in sizes:
        out.append((c0, s))
        c0 += s
    return out


def build_nc(N=FULL_N, D=FULL_D, cs=CS):
    """Build the per-core Bass graph. Same graph on all 8 cores (SPMD).

    Inputs:  xnt [D, N] bf16 (normalized-x transposed, full batch)
             w [D, cs] bf16 (transposed class shard)
    Outputs: out [N, cs] bf16 (wf shard), sums [128, N//128] f32
             (sums[p, m] = sum_c exp(S * wf[m*128+p, c]))
    """
    NM = N // P  # row tiles
    KD = D // P  # contraction tiles
    NCB = cs // CB  # class blocks per core
    chunks = _chunks(NCB)

    nc = bacc.Bacc(trn_type="TRN2", target_bir_lowering=False, debug=False)
    xnt_ext = nc.dram_tensor("xnt", [D, N], BF16, kind="ExternalInput").ap()
    w_ext = nc.dram_tensor("w", [D, cs], BF16, kind="ExternalInput").ap()
    out_ext = nc.dram_tensor("out", [N, cs], BF16, kind="ExternalOutput").ap()
    sums_ext = nc.dram_tensor("sums", [P, NM], F32, kind="ExternalOutput").ap()
    out_v = out_ext.rearrange("(m p) c -> p m c", p=P)

    with tile.TileContext(nc) as tc:
        with (
            tc.tile_pool(name="xp", bufs=1) as xpool,
            tc.tile_pool(name="stats", bufs=1) as stats,
            tc.tile_pool(name="wt", bufs=3) as wt_pool,
            tc.tile_pool(name="wfout", bufs=3) as wf_pool,
            tc.tile_pool(name="expsc", bufs=3) as exp_pool,
            tc.tile_pool(name="psmm", bufs=4, space="PSUM") as psum_mm,
        ):
            xnT = xpool.tile([P, KD, N], BF16, tag="xnT")
            xnt_v = xnt_ext.rearrange("(j p) i -> p j i", p=P)

            def _load_xnt(j):
                nc.sync.dma_start(
                    out=xnT[:, j : j + 1, :], in_=xnt_v[:, j : j + 1, :]
                )

            _load_xnt(0)

            # partial exp-sums: one slot per (row-tile, super-chunk)
            sums_acc = stats.tile([P, NM, len(chunks)], F32, tag="sums_acc")

            for ci, (cb0, nq) in enumerate(chunks):
                W = nq * CB
                c0 = cb0 * CB
                wt = wt_pool.tile([P, KD, QB * CB], BF16, tag="wt")
                w_v = w_ext[:, c0 : c0 + W].rearrange("(j p) c -> p j c", p=P)
                if ci == 0:
                    nc.sync.dma_start(
                        out=wt[:, 0:1, :W], in_=w_v[:, 0:1, :]
                    )
                    _load_xnt(1)
                    for j in range(1, KD):
                        nc.sync.dma_start(
                            out=wt[:, j : j + 1, :W], in_=w_v[:, j : j + 1, :]
                        )
                        if j + 1 < KD:
                            _load_xnt(j + 1)
                else:
                    nc.sync.dma_start(out=wt[:, :, :W], in_=w_v)

                wf_big = wf_pool.tile([P, NM, QB * CB], BF16, tag="wfbig")
                nh = QB // 2  # blocks per psum-half tile (2 banks)
                for m in range(NM):
                    for h0 in range(0, nq, nh):
                        hq = min(nh, nq - h0)
                        pmm = psum_mm.tile([P, nh * CB], F32, tag="psmm")
                        for q in range(h0, h0 + hq):
                            for j in range(KD):
                                nc.tensor.matmul(
                                    pmm[:, (q - h0) * CB : (q - h0 + 1) * CB],
                                    xnT[:, j, m * P : (m + 1) * P],
                                    wt[:, j, q * CB : (q + 1) * CB],
                                    start=(j == 0),
                                    stop=(j == KD - 1),
                                )
                        for q in range(h0, h0 + hq):
                            nc.vector.tensor_copy(
                                wf_big[:, m, q * CB : (q + 1) * CB],
                                pmm[:, (q - h0) * CB : (q - h0 + 1) * CB],
                            )
                    es = exp_pool.tile([P, QB * CB], BF16, tag="es")
                    nc.scalar.activation(
                        out=es[:, :W],
                        in_=wf_big[:, m, :W],
                        func=AF.Exp,
                        scale=S_SCALE,
                        accum_out=sums_acc[:, m, ci : ci + 1],
                    )
                    nc.scalar.dma_start(
                        out=out_v[:, m, c0 : c0 + W], in_=wf_big[:, m, :W]
                    )

            # ---- final partial-sum reduce + store ----
            sums_red = stats.tile([P, NM], F32, tag="sums_red")
            nc.vector.tensor_reduce(
                out=sums_red[:],
                in_=sums_acc[:],
                axis=mybir.AxisListType.X,
                op=mybir.AluOpType.add,
            )
            nc.scalar.dma_start(out=sums_ext[:, :], in_=sums_red[:])

    nc.compile()
    return nc


_NC_CACHE = {}


def _get_nc(N, D, cs):
    key = (N, D, cs)
    if key not in _NC_CACHE:
        _NC_CACHE[key] = build_nc(N, D, cs)
    return _NC_CACHE[key]


def run_device(xnt, wt_shards, N, D, cs, trace=False):
    """Run the SPMD kernel.

    xnt: [D, N] bf16 (normalized x, transposed; same on all cores)
    wt_shards: per-core [D, cs] bf16 (pre-transposed class shards)
    Returns (wf_shards bf16, sums_shards f32, results_obj)."""
    nc = _get_nc(N, D, cs)
    in_maps = [{"xnt": xnt, "w": ws} for ws in wt_shards]
    res = run_bass_kernel_spmd(
        nc, in_maps, core_ids=list(range(N_CORES)), trace=trace
    )
    wf_shards = [r["out"] for r in res.results]
    sums_shards = [r["sums"] for r in res.results]
    return wf_shards, sums_shards, res


def _make_shards(W, D, cs):
    C = W.shape[0]
    c_pad = N_CORES * cs
    w_bf = np.asarray(W, dtype=ml_dtypes.bfloat16)
    shards = []
    for i in range(N_CORES):
        lo, hi = i * cs, min((i + 1) * cs, C)
        st = np.zeros((cs, D), dtype=ml_dtypes.bfloat16)
        st[: hi - lo] = w_bf[lo:hi]
        shards.append(np.ascontiguousarray(st.T))
    return shards, c_pad - C


def _normalize_transpose(x):
    x = np.asarray(x, dtype=np.float32)
    nrm = np.maximum(np.sqrt((x * x).sum(axis=1, keepdims=True)), 1e-12)
    xn = x / nrm
    return np.ascontiguousarray(xn.T.astype(ml_dtypes.bfloat16))


def kernel(x, W, labels, positive, _trace=False, _ret_res=False):
    N, D = x.shape
    C = W.shape[0]
    assert (N, D, C) == (FULL_N, FULL_D, FULL_C), (N, D, C)

    cs = CS
    wt_shards, n_pad = _make_shards(W, D, cs)
    xnt = _normalize_transpose(x)

    wf_shards, sums_shards, res = run_device(
        xnt, wt_shards, N, D, cs, trace=_trace
    )

    wf = np.concatenate(
        [np.asarray(s, dtype=np.float32) for s in wf_shards], axis=1
    )[:, :C]

    if not int(positive):
        return (wf, res) if _ret_res else wf

    # sums[p, m] holds row i = m*128 + p; pads contribute exp(0) = 1 each
    NM = N // P
    total = np.zeros((P, NM), dtype=np.float64)
    for s in sums_shards:
        total += s.astype(np.float64)
    total_rows = total.T.reshape(N)  # index i = m*128 + p
    total_rows = total_rows - float(n_pad)

    labels = np.asarray(labels).astype(np.int64)
    rows = np.arange(N)
    target = wf[rows, labels].astype(np.float64)
    tgt = np.clip(target, -1.0 + CLIP_EPS, 1.0 - CLIP_EPS)
    numerator = S_SCALE * np.cos(np.arccos(tgt) + MARGIN)
    excl = total_rows - np.exp(S_SCALE * target)
    denom = np.exp(numerator) + excl
    L = numerator - np.log(denom)
    loss = np.float32(-np.mean(L))

    out = (wf, loss)
    return (out, res) if _ret_res else out


if __name__ == "__main__":
    # smoke test at reduced dims: validates every op + SPMD plumbing
    np.random.seed(0)
    N, D, cs = 256, 256, 1024
    x = np.random.randn(N, D).astype(np.float32)
    w_shards = [
        (np.random.randn(cs, D) * 0.05).astype(np.float32)
        for _ in range(N_CORES)
    ]
    wt_shards = [
        np.ascontiguousarray(np.asarray(w, dtype=ml_dtypes.bfloat16).T)
        for w in w_shards
    ]
    xnt = _normalize_transpose(x)
    wf_shards, sums_shards, _ = run_device(xnt, wt_shards, N, D, cs)

    xn = x / np.maximum(np.linalg.norm(x, axis=1, keepdims=True), 1e-12)
    max_err = 0.0
    for c in range(N_CORES):
        exp_wf = xn @ w_shards[c].T
        got = np.asarray(wf_shards[c], dtype=np.float32)
        e = np.abs(got - exp_wf).max() / max(np.abs(exp_wf).max(), 1e-9)
        max_err = max(max_err, e)
        exp_sums = np.exp(S_SCALE * exp_wf).sum(axis=1)  # [N]
        got_sums = sums_shards[c].T.reshape(N)
        es = np.abs(got_sums - exp_sums) / np.abs(exp_sums)
        print(f"core {c}: wf_rel={e:.2e} sums_rel_max={es.max():.2e}")
    print("smoke max wf rel err:", max_err)
    assert max_err < 2e-2, max_err
    print("SMOKE PASSED")


# revision 17
# speedup vs baseline: 1.0169x; 1.0169x over previous
"""ArcFace (AngularPenaltySMLoss) Trainium2 kernel.

Computes, for x [N, D], W [C, D], labels [N]:
  xn = x / max(||x||_2, 1e-12)   (row-normalize)
  wf = xn @ W.T                  [N, C]
  target = wf[i, labels[i]]
  numerator = S * cos(arccos(clip(target)) + M)
  L = numerator - log(exp(numerator) + sum_{j != label} exp(S * wf[i, j]))
  returns (wf, -mean(L))

Distribution: W is sharded over the class dim C across 8 NeuronCores
(tensor-parallel margin-softmax). Each core computes its wf shard plus
per-row sums of exp(S*wf) over its shard; the [N]-sized combination
(label gather, margin, log) runs on host.

Layout/precision strategy:
  - Host normalizes x (f32, same formula as the reference), casts to
    bf16 and pre-transposes to xnT [D, N]; host also pre-transposes +
    bf16-casts each W shard to wT [D, CS]. The device then needs zero
    transposes: the contraction dim D lands on partitions from DMA.
  - Matmuls run bf16 x bf16 -> f32 PSUM at full PE rate (fp32r is a
    half-rate LOW_HIGH two-pass mode on trn2; fp32 is quarter-rate).
  - Class blocks are processed 4-at-a-time (2048 classes / 4 PSUM
    banks) so the PSUM->bf16 cast (DVE) and exp+row-sum (ACT) pay
    their fixed per-instruction cost once per 2048 elements.
  - wf is stored/DMA'd as bf16 (the output gate is scale-relative
    2e-2; bf16 rounding lands ~4e-3 max here). exp(64*wf) runs on the
    scalar engine from the bf16 tile at full 16-bit rate with per-row
    f32 accumulation; the [1024]-sized finishing math runs on host.
"""

import os
import sys

import numpy as np

for _p in ("/opt/trn_rl_repo", "/root/.axon_site/_ro/trn_rl_repo"):
    if os.path.isdir(_p) and _p not in sys.path:
        sys.path.insert(0, _p)

import ml_dtypes

import concourse.bass as bass
import concourse.tile as tile
from concourse import bacc, mybir
from concourse.bass_utils import run_bass_kernel_spmd

AF = mybir.ActivationFunctionType
F32 = mybir.dt.float32
BF16 = mybir.dt.bfloat16

S_SCALE = 64.0
MARGIN = 0.5
CLIP_EPS = 1e-7

N_CORES = 8
P = 128
CB = 512  # class-block width (one PSUM bank of fp32)
QB = 4  # class blocks per super-chunk (4 PSUM banks)

# Problem dims (hardcoded; kernel() asserts against them)
FULL_N, FULL_D, FULL_C = 1024, 512, 85742
CS = 10752  # per-core class shard, padded: 8 * 10752 = 86016 = 21 blocks of 512


def _chunks(ncb):
    # First chunk small-ish (its W DMA gates the very first matmul),
    # tail chunk smaller than QB (shorter exp on the drain chain).
    sizes = []
    first = min(2, ncb)
    sizes.append(first)
    rem = ncb - first
    while rem > 0:
        if rem > QB:
            sizes.append(QB)
            rem -= QB
        else:
            sizes.append(rem)
            rem = 0
    out = []
    c0 = 0
    for s in sizes:
        out.append((c0, s))
        c0 += s
    return out


def build_nc(N=FULL_N, D=FULL_D, cs=CS):
    """Build the per-core Bass graph. Same graph on all 8 cores (SPMD).

    Inputs:  xnt [D, N] bf16 (normalized-x transposed, full batch)
             w [D, cs] bf16 (transposed class shard)
    Outputs: out [N, cs] bf16 (wf shard), sums [128, N//128] f32
             (sums[p, m] = sum_c exp(S * wf[m*128+p, c]))
    """
    NM = N // P  # row tiles
    KD = D // P  # contraction tiles
    NCB = cs // CB  # class blocks per core
    chunks = _chunks(NCB)

    nc = bacc.Bacc(trn_type="TRN2", target_bir_lowering=False, debug=False)
    xnt_ext = nc.dram_tensor("xnt", [D, N], BF16, kind="ExternalInput").ap()
    w_ext = nc.dram_tensor("w", [D, cs], BF16, kind="ExternalInput").ap()
    out_ext = nc.dram_tensor("out", [N, cs], BF16, kind="ExternalOutput").ap()
    sums_ext = nc.dram_tensor("sums", [P, NM], F32, kind="ExternalOutput").ap()
    out_v = out_ext.rearrange("(m p) c -> p m c", p=P)

    with tile.TileContext(nc) as tc:
        with (
            tc.tile_pool(name="xp", bufs=1) as xpool,
            tc.tile_pool(name="stats", bufs=1) as stats,
            tc.tile_pool(name="wt", bufs=4) as wt_pool,
            tc.tile_pool(name="wfout", bufs=3) as wf_pool,
            tc.tile_pool(name="expsc", bufs=2) as exp_pool,
            tc.tile_pool(name="psmm", bufs=4, space="PSUM") as psum_mm,
        ):
            xnT = xpool.tile([P, KD, N], BF16, tag="xnT")
            xnt_v = xnt_ext.rearrange("(j p) i -> p j i", p=P)

            def _load_xnt(j):
                nc.sync.dma_start(
                    out=xnT[:, j : j + 1, :], in_=xnt_v[:, j : j + 1, :]
                )

            _load_xnt(0)

            # partial exp-sums: one slot per (row-tile, super-chunk)
            sums_acc = stats.tile([P, NM, 2 * len(chunks)], F32, tag="sums_acc")

            for ci, (cb0, nq) in enumerate(chunks):
                W = nq * CB
                c0 = cb0 * CB
                wt = wt_pool.tile([P, KD, QB * CB], BF16, tag="wt")
                w_v = w_ext[:, c0 : c0 + W].rearrange("(j p) c -> p j c", p=P)
                if ci == 0:
                    nc.sync.dma_start(
                        out=wt[:, 0:1, :W], in_=w_v[:, 0:1, :]
                    )
                    _load_xnt(1)
                    for j in range(1, KD):
                        nc.sync.dma_start(
                            out=wt[:, j : j + 1, :W], in_=w_v[:, j : j + 1, :]
                        )
                        if j + 1 < KD:
                            _load_xnt(j + 1)
                else:
                    nc.sync.dma_start(out=wt[:, :, :W], in_=w_v)

                wf_big = wf_pool.tile([P, NM, QB * CB], BF16, tag="wfbig")
                nh = QB // 2  # blocks per psum-half tile (2 banks)
                for m in range(NM):
                    for h0 in range(0, nq, nh):
                        hq = min(nh, nq - h0)
                        pmm = psum_mm.tile([P, nh * CB], F32, tag="psmm")
                        for q in range(h0, h0 + hq):
                            for j in range(KD):
                                nc.tensor.matmul(
                                    pmm[:, (q - h0) * CB : (q - h0 + 1) * CB],
                                    xnT[:, j, m * P : (m + 1) * P],
                                    wt[:, j, q * CB : (q + 1) * CB],
                                    start=(j == 0),
                                    stop=(j == KD - 1),
                                )
                        for q in range(h0, h0 + hq):
                            nc.vector.tensor_copy(
                                wf_big[:, m, q * CB : (q + 1) * CB],
                                pmm[:, (q - h0) * CB : (q - h0 + 1) * CB],
                            )
                        es = exp_pool.tile([P, QB * CB], BF16, tag="es")
                        hslot = 2 * ci + h0 // nh
                        nc.scalar.activation(
                            out=es[:, : hq * CB],
                            in_=wf_big[:, m, h0 * CB : (h0 + hq) * CB],
                            func=AF.Exp,
                            scale=S_SCALE,
                            accum_out=sums_acc[:, m, hslot : hslot + 1],
                        )
                    nc.scalar.dma_start(
                        out=out_v[:, m, c0 : c0 + W], in_=wf_big[:, m, :W]
                    )

            # ---- final partial-sum reduce + store ----
            sums_red = stats.tile([P, NM], F32, tag="sums_red")
            nc.vector.tensor_reduce(
                out=sums_red[:],
                in_=sums_acc[:],
                axis=mybir.AxisListType.X,
                op=mybir.AluOpType.add,
            )
            nc.scalar.dma_start(out=sums_ext[:, :], in_=sums_red[:])

    nc.compile()
    return nc


_NC_CACHE = {}


def _get_nc(N, D, cs):
    key = (N, D, cs)
    if key not in _NC_CACHE:
        _NC_CACHE[key] = build_nc(N, D, cs)
    return _NC_CACHE[key]


def run_device(xnt, wt_shards, N, D, cs, trace=False):
    """Run the SPMD kernel.

    xnt: [D, N] bf16 (normalized x, transposed; same on all cores)
    wt_shards: per-core [D, cs] bf16 (pre-transposed class shards)
    Returns (wf_shards bf16, sums_shards f32, results_obj)."""
    nc = _get_nc(N, D, cs)
    in_maps = [{"xnt": xnt, "w": ws} for ws in wt_shards]
    res = run_bass_kernel_spmd(
        nc, in_maps, core_ids=list(range(N_CORES)), trace=trace
    )
    wf_shards = [r["out"] for r in res.results]
    sums_shards = [r["sums"] for r in res.results]
    return wf_shards, sums_shards, res


def _make_shards(W, D, cs):
    C = W.shape[0]
    c_pad = N_CORES * cs
    w_bf = np.asarray(W, dtype=ml_dtypes.bfloat16)
    shards = []
    for i in range(N_CORES):
        lo, hi = i * cs, min((i + 1) * cs, C)
        st = np.zeros((cs, D), dtype=ml_dtypes.bfloat16)
        st[: hi - lo] = w_bf[lo:hi]
        shards.append(np.ascontiguousarray(st.T))
    return shards, c_pad - C


def _normalize_transpose(x):
    x = np.asarray(x, dtype=np.float32)
    nrm = np.maximum(np.sqrt((x * x).sum(axis=1, keepdims=True)), 1e-12)
    xn = x / nrm
    return np.ascontiguousarray(xn.T.astype(ml_dtypes.bfloat16))


def kernel(x, W, labels, positive, _trace=False, _ret_res=False):
    N, D = x.shape
    C = W.shape[0]
    assert (N, D, C) == (FULL_N, FULL_D, FULL_C), (N, D, C)

    cs = CS
    wt_shards, n_pad = _make_shards(W, D, cs)
    xnt = _normalize_transpose(x)

    wf_shards, sums_shards, res = run_device(
        xnt, wt_shards, N, D, cs, trace=_trace
    )

    wf = np.concatenate(
        [np.asarray(s, dtype=np.float32) for s in wf_shards], axis=1
    )[:, :C]

    if not int(positive):
        return (wf, res) if _ret_res else wf

    # sums[p, m] holds row i = m*128 + p; pads contribute exp(0) = 1 each
    NM = N // P
    total = np.zeros((P, NM), dtype=np.float64)
    for s in sums_shards:
        total += s.astype(np.float64)
    total_rows = total.T.reshape(N)  # index i = m*128 + p
    total_rows = total_rows - float(n_pad)

    labels = np.asarray(labels).astype(np.int64)
    rows = np.arange(N)
    target = wf[rows, labels].astype(np.float64)
    tgt = np.clip(target, -1.0 + CLIP_EPS, 1.0 - CLIP_EPS)
    numerator = S_SCALE * np.cos(np.arccos(tgt) + MARGIN)
    excl = total_rows - np.exp(S_SCALE * target)
    denom = np.exp(numerator) + excl
    L = numerator - np.log(denom)
    loss = np.float32(-np.mean(L))

    out = (wf, loss)
    return (out, res) if _ret_res else out


if __name__ == "__main__":
    # smoke test at reduced dims: validates every op + SPMD plumbing
    np.random.seed(0)
    N, D, cs = 256, 256, 1024
    x = np.random.randn(N, D).astype(np.float32)
    w_shards = [
        (np.random.randn(cs, D) * 0.05).astype(np.float32)
        for _ in range(N_CORES)
    ]
    wt_shards = [
        np.ascontiguousarray(np.asarray(w, dtype=ml_dtypes.bfloat16).T)
        for w in w_shards
    ]
    xnt = _normalize_transpose(x)
    wf_shards, sums_shards, _ = run_device(xnt, wt_shards, N, D, cs)

    xn = x / np.maximum(np.linalg.norm(x, axis=1, keepdims=True), 1e-12)
    max_err = 0.0
    for c in range(N_CORES):
        exp_wf = xn @ w_shards[c].T
        got = np.asarray(wf_shards[c], dtype=np.float32)
        e = np.abs(got - exp_wf).max() / max(np.abs(exp_wf).max(), 1e-9)
        max_err = max(max_err, e)
        exp_sums = np.exp(S_SCALE * exp_wf).sum(axis=1)  # [N]
        got_sums = sums_shards[c].T.reshape(N)
        es = np.abs(got_sums - exp_sums) / np.abs(exp_sums)
        print(f"core {c}: wf_rel={e:.2e} sums_rel_max={es.max():.2e}")
    print("smoke max wf rel err:", max_err)
    assert max_err < 2e-2, max_err
    print("SMOKE PASSED")
